# revision 3
# baseline (speedup 1.0000x reference)
"""Hardest-positive / hardest-negative mining kernel for Trainium2.

Problem: per domain d (4 domains), emb [8192,128], labels [8192] in [0,64):
  sim = emb @ emb.T; hardest positive = min same-label sim (excl self),
  hardest negative = max diff-label sim; plus the corresponding embeddings.

Strategy (8 NeuronCores, SPMD single program):
  - core c handles domain c//2, row-half c%2 (4096 rows), rows/cols sorted
    by label on host so positives are contiguous segments.
  - Exact-enough sims via split-precision fp32r matmuls:
    emb = h + l (both 11-bit-mantissa fp32r), sim ~= h.h + h.l + l.h
    (error ~3e-5, measured decision margins are ~2e-4).
  - Negatives: max8 top-8 over the full row + max_index (equality search),
    then filter out same-label/self entries using per-row segment bounds.
  - Positives: a 448-wide host-gathered window around each row-tile covers
    every row's own segment; an extra -256*onehot matmul marks same-label
    columns, min over window lands on a marked column; value = min + 256.
  - Embedding rows fetched by indirect DMA from DRAM.
  - Host unsorts and assembles full outputs.
"""

import sys

sys.path.insert(0, "/opt/trn_rl_repo")

import numpy as np

import concourse.bass as bass
import concourse.bacc as bacc
import concourse.mybir as mybir
from concourse import tile
from concourse.bass_utils import run_bass_kernel_spmd

D, B, F, NL = 4, 8192, 128, 64
HALF = B // 2            # rows per core
NT = HALF // 128         # 32 row tiles per core
NCH = B // 512           # 16 column chunks
W = 448                  # positive-mining window width
WOFF = 160               # window starts at rowbase - WOFF

f32 = mybir.dt.float32
f32r = mybir.dt.float32r
bf16 = mybir.dt.bfloat16
u32 = mybir.dt.uint32
i32 = mybir.dt.int32
Alu = mybir.AluOpType
Ax = mybir.AxisListType


def _round_fp32r(a: np.ndarray) -> np.ndarray:
    """Round fp32 to 11-bit mantissa (rne to 20-bit [s|e8|m11])."""
    u = a.astype(np.float32).view(np.uint32).astype(np.uint64)
    u = (u + 0x7FF + ((u >> 12) & 1)) & np.uint64(0xFFFFF000)
    return u.astype(np.uint32).view(np.float32)


_PROGRAM = None


def _build_program():
    global _PROGRAM
    if _PROGRAM is not None:
        return _PROGRAM
    nc = bacc.Bacc("TRN2", target_bir_lowering=False, debug=False, num_devices=8)

    hT = nc.dram_tensor("hT", [F, B], f32r, kind="ExternalInput").ap()
    lT = nc.dram_tensor("lT", [F, B], f32r, kind="ExternalInput").ap()
    hTr = nc.dram_tensor("hTr", [F, HALF], f32r, kind="ExternalInput").ap()
    lTr = nc.dram_tensor("lTr", [F, HALF], f32r, kind="ExternalInput").ap()
    ohnr = nc.dram_tensor("ohnr", [NL, HALF], bf16, kind="ExternalInput").ap()
    winh = nc.dram_tensor("winh", [F, NT, W], f32r, kind="ExternalInput").ap()
    winl = nc.dram_tensor("winl", [F, NT, W], f32r, kind="ExternalInput").ap()
    winp = nc.dram_tensor("winp", [NL, NT, W], bf16, kind="ExternalInput").ap()
    segb = nc.dram_tensor("segb", [128, 2 * NT], f32, kind="ExternalInput").ap()
    wbase = nc.dram_tensor("wbase", [128, NT], f32, kind="ExternalInput").ap()

    pd_o = nc.dram_tensor("pd", [128, NT], f32, kind="ExternalOutput").ap()
    nd_o = nc.dram_tensor("nd", [128, NT], f32, kind="ExternalOutput").ap()
    pi_o = nc.dram_tensor("pi", [128, NT], f32, kind="ExternalOutput").ap()
    ni_o = nc.dram_tensor("ni", [128, NT], f32, kind="ExternalOutput").ap()

    with tile.TileContext(nc) as tc:
        with (
            tc.tile_pool(name="emb", bufs=1) as embp,
            tc.tile_pool(name="xp", bufs=2) as xp,
            tc.tile_pool(name="ps", bufs=6, space="PSUM") as ps,
            tc.tile_pool(name="psw", bufs=2, space="PSUM") as psw,
            tc.tile_pool(name="wt", bufs=3) as wt,
            tc.tile_pool(name="sm", bufs=3) as sm,
            tc.tile_pool(name="res", bufs=1) as res,
        ):
            ht = embp.tile([F, B], f32r)
            nc.sync.dma_start(ht[:], hT[:])
            lt = embp.tile([F, B], f32r)
            nc.sync.dma_start(lt[:], lT[:])
            htr = embp.tile([F, HALF], f32r)
            nc.sync.dma_start(htr[:], hTr[:])
            ltr = embp.tile([F, HALF], f32r)
            nc.sync.dma_start(ltr[:], lTr[:])
            ohr = embp.tile([NL, HALF], bf16)
            nc.sync.dma_start(ohr[:], ohnr[:])
            segt = embp.tile([128, 2 * NT], f32)
            nc.sync.dma_start(segt[:], segb[:])
            wbt = embp.tile([128, NT], f32)
            nc.sync.dma_start(wbt[:], wbase[:])

            pd_b = res.tile([128, NT], f32)
            nd_b = res.tile([128, NT], f32)
            pidx_b = res.tile([128, NT], f32)
            nidx_b = res.tile([128, NT], f32)

            for t in range(NT):
                rs = slice(128 * t, 128 * t + 128)
                # ---- full-row sims: X[p, j] = emb_row(p) . emb_col(j)
                X = xp.tile([128, B], f32, tag="X")
                for c in range(NCH):
                    pt = ps.tile([128, 512], f32, tag="pt")
                    cs = slice(c * 512, (c + 1) * 512)
                    nc.tensor.matmul(pt[:], htr[:, rs], ht[:, cs], start=True, stop=False)
                    nc.tensor.matmul(pt[:], htr[:, rs], lt[:, cs], start=False, stop=False)
                    nc.tensor.matmul(pt[:], ltr[:, rs], ht[:, cs], start=False, stop=True)
                    nc.scalar.copy(X[:, cs], pt[:])

                # ---- negatives: top-8 values + indices, filter same-label
                m8 = sm.tile([128, 8], f32, tag="m8")
                nc.vector.max(m8[:], X[:])
                i8 = sm.tile([128, 8], u32, tag="i8")
                nc.vector.max_index(i8[:], m8[:], X[:])
                i8f = sm.tile([128, 8], f32, tag="i8f")
                nc.vector.tensor_copy(i8f[:], i8[:])
                lo = sm.tile([128, 8], f32, tag="lo")
                nc.vector.tensor_scalar(
                    lo[:], i8f[:], segt[:, 2 * t : 2 * t + 1], None, op0=Alu.is_lt
                )
                hi = sm.tile([128, 8], f32, tag="hi")
                nc.vector.tensor_scalar(
                    hi[:], i8f[:], segt[:, 2 * t + 1 : 2 * t + 2], None, op0=Alu.is_ge
                )
                valid = sm.tile([128, 8], f32, tag="valid")
                nc.vector.tensor_add(valid[:], lo[:], hi[:])
                vm8 = sm.tile([128, 8], f32, tag="vm8")
                # masked = valid*m8 + (valid-1)*1e30  (invalid -> ~-1e30)
                nc.vector.tensor_tensor(vm8[:], m8[:], valid[:], op=Alu.mult)
                vpen = sm.tile([128, 8], f32, tag="vpen")
                nc.vector.tensor_scalar(
                    vpen[:], valid[:], 1.0, 1e30, op0=Alu.subtract, op1=Alu.mult
                )
                nc.vector.tensor_add(vm8[:], vm8[:], vpen[:])
                nc.vector.tensor_reduce(
                    out=nd_b[:, t : t + 1], in_=vm8[:], op=Alu.max, axis=Ax.X
                )
                eq8 = sm.tile([128, 8], f32, tag="eq8")
                nc.vector.tensor_scalar(
                    eq8[:], vm8[:], nd_b[:, t : t + 1], None, op0=Alu.is_equal
                )
                nc.vector.tensor_tensor(eq8[:], eq8[:], i8f[:], op=Alu.mult)
                nc.vector.tensor_reduce(
                    out=nidx_b[:, t : t + 1], in_=eq8[:], op=Alu.max, axis=Ax.X
                )

                # ---- positives: window sims + (-256)*same-label marker
                XW = xp.tile([128, W], f32, tag="XW")
                wp = psw.tile([128, W], f32, tag="wp")
                wh = wt.tile([F, W], f32r, tag="wh")
                nc.sync.dma_start(wh[:], winh[:, t, :])
                wl = wt.tile([F, W], f32r, tag="wl")
                nc.sync.dma_start(wl[:], winl[:, t, :])
                wo = wt.tile([NL, W], bf16, tag="wo")
                nc.sync.dma_start(wo[:], winp[:, t, :])
                nc.tensor.matmul(wp[:], htr[:, rs], wh[:], start=True, stop=False)
                nc.tensor.matmul(wp[:], htr[:, rs], wl[:], start=False, stop=False)
                nc.tensor.matmul(wp[:], ltr[:, rs], wh[:], start=False, stop=False)
                nc.tensor.matmul(wp[:], ohr[:, rs], wo[:], start=False, stop=True)
                nc.scalar.copy(XW[:], wp[:])
                minw = sm.tile([128, 1], f32, tag="minw")
                nc.vector.tensor_reduce(out=minw[:], in_=XW[:], op=Alu.min, axis=Ax.X)
                nc.vector.tensor_scalar(
                    pd_b[:, t : t + 1], minw[:], 256.0, None, op0=Alu.add
                )
                m1w = sm.tile([128, 8], f32, tag="m1w")
                nc.vector.tensor_copy(m1w[:], minw[:].to_broadcast([128, 8]))
                iw = sm.tile([128, 8], u32, tag="iw")
                nc.vector.max_index(iw[:], m1w[:], XW[:])
                iwf = sm.tile([128, 1], f32, tag="iwf")
                nc.vector.tensor_copy(iwf[:], iw[:, 0:1])
                nc.vector.tensor_scalar(
                    pidx_b[:, t : t + 1],
                    iwf[:],
                    wbt[:, t : t + 1],
                    None,
                    op0=Alu.add,
                )

            nc.sync.dma_start(pi_o[:], pidx_b[:])
            nc.sync.dma_start(ni_o[:], nidx_b[:])
            nc.sync.dma_start(pd_o[:], pd_b[:])
            nc.sync.dma_start(nd_o[:], nd_b[:])

    nc.finalize()
    _PROGRAM = nc
    return nc


def kernel(semb: np.ndarray, slabels: np.ndarray, _want_trace: bool = False):
    semb = np.asarray(semb, dtype=np.float32)
    slabels_in = np.asarray(slabels)
    lab = slabels_in.astype(np.int64)

    nc = _build_program()

    in_maps = []
    orders = []
    for c in range(8):
        d, half = c // 2, c % 2
        emb = semb[d]
        order = np.argsort(lab[d], kind="stable")
        orders.append(order)
        emb_s = np.ascontiguousarray(emb[order])
        lab_s = lab[d][order]
        h = _round_fp32r(emb_s)
        l = _round_fp32r(emb_s - h)
        hT = np.ascontiguousarray(h.T)
        lT = np.ascontiguousarray(l.T)
        r0 = half * HALF
        hTr = np.ascontiguousarray(hT[:, r0 : r0 + HALF])
        lTr = np.ascontiguousarray(lT[:, r0 : r0 + HALF])

        oh = np.zeros((NL, B), np.float32)
        oh[lab_s, np.arange(B)] = 1.0
        import ml_dtypes

        ohn = (-256.0 * oh).astype(ml_dtypes.bfloat16)
        ohnr = np.ascontiguousarray(ohn[:, r0 : r0 + HALF])

        # per-row segment bounds in sorted space
        counts = np.bincount(lab_s, minlength=NL)
        starts = np.concatenate([[0], np.cumsum(counts)[:-1]])
        ends = np.cumsum(counts)
        segstart = starts[lab_s]  # [B]
        segend = ends[lab_s]

        # windows
        winh = np.zeros((F, NT, W), np.float32)
        winl = np.zeros((F, NT, W), np.float32)
        winp = np.zeros((NL, NT, W), np.float32)
        ohp = oh.astype(np.float32)
        wbs = np.zeros(NT, np.int64)
        for t in range(NT):
            rowbase = r0 + 128 * t
            wb = rowbase - WOFF
            wbs[t] = wb
            a, b = max(wb, 0), min(wb + W, B)
            winh[:, t, a - wb : b - wb] = hT[:, a:b]
            winl[:, t, a - wb : b - wb] = lT[:, a:b]
            winp[:, t, a - wb : b - wb] = ohp[:, a:b]
            # invariant: every row's segment fits in its window
            g0, g1 = rowbase, rowbase + 128
            assert segstart[g0:g1].min() >= wb and segend[g0:g1].max() <= wb + W

        segb = np.zeros((128, 2 * NT), np.float32)
        wbase = np.zeros((128, NT), np.float32)
        for t in range(NT):
            rows = np.arange(r0 + 128 * t, r0 + 128 * t + 128)
            segb[:, 2 * t] = segstart[rows]
            segb[:, 2 * t + 1] = segend[rows]
            wbase[:, t] = wbs[t]

        emb_sorted_cache = emb_s
        in_maps.append(
            {
                "hT": hT,
                "lT": lT,
                "hTr": hTr,
                "lTr": lTr,
                "ohnr": ohnr,
                "winh": winh.astype(np.float32),
                "winl": winl.astype(np.float32),
                "winp": winp.astype(ml_dtypes.bfloat16),
                "segb": segb,
                "wbase": wbase,
            }
        )

    import time as _time

    _t0 = _time.time()
    res = run_bass_kernel_spmd(nc, in_maps, core_ids=list(range(8)))
    kernel._last_run_s = _time.time() - _t0

    pos_dist = np.zeros((D, B), np.float32)
    neg_dist = np.zeros((D, B), np.float32)
    pos_embed = np.zeros((D, B, F), np.float32)
    neg_embed = np.zeros((D, B, F), np.float32)
    embs_by_domain = {}
    for c in range(8):
        d = c // 2
        if d not in embs_by_domain:
            embs_by_domain[d] = np.ascontiguousarray(semb[d][orders[c]])
    for c in range(8):
        d, half = c // 2, c % 2
        r = res.results[c]
        order = orders[c]
        emb_s = embs_by_domain[d]
        # device row (p, t) -> sorted row half*HALF + 128 t + p -> orig order[g]
        g = half * HALF + 128 * np.arange(NT)[None, :] + np.arange(128)[:, None]
        rows = order[g]  # [128, NT]
        pos_dist[d, rows] = r["pd"]
        neg_dist[d, rows] = r["nd"]
        pos_embed[d, rows] = emb_s[r["pi"].astype(np.int64)]
        neg_embed[d, rows] = emb_s[r["ni"].astype(np.int64)]

    return pos_dist, neg_dist, pos_embed, neg_embed
